# revision 17
# baseline (speedup 1.0000x reference)
import sys

sys.path.insert(0, "/opt/trn_rl_repo")

import numpy as np
import ml_dtypes
import concourse.bass as bass  # noqa: F401  (registers types)
from concourse import bacc
import concourse.mybir as mybir
from concourse.tile import TileContext
from concourse.bass_utils import run_bass_kernel_spmd

S = 4096          # sequence length
D = 1024          # model/key/value dim
NCORES = 8
R = S // NCORES   # 512 rows per core
KK = 4            # 256-deep contraction double-chunks (DoubleRow)
J = S // 128      # 32 key tiles
VA = D + 2        # V augmented with ones column (denominator) + zero pad
CH = VA // 3      # 342-wide PV output chunks (3 chunks, one PSUM bank each)
WSC = np.float32(16.0)   # fp8 prescale for projection weights
# scores width for key tile j under 16-row strip interleaving: strips k<=j
# plus one fully-masked strip so PV key-pair reads stay in computed territory
W_J = [min(R, 16 * (j + 2) if j % 2 == 0 else 16 * (j + 1)) for j in range(J)]

F32 = mybir.dt.float32
BF16 = mybir.dt.bfloat16
E4 = mybir.dt.float8e4
DR = mybir.MatmulPerfMode.DoubleRow
E4NP = ml_dtypes.float8_e4m3fn

_cache = {}


def _warmup(nc, pool, psum_pool, tag, n):
    if n == 0:
        return
    wz = pool.tile([128, 2, 64], E4, name="wz")
    nc.vector.memset(wz[:], 0)
    wps = psum_pool.tile([128, 512], F32, name="wps", tag=tag)
    for _ in range(n):
        nc.tensor.matmul(wps[0:64, 0:64], wz[:, :, 0:64], wz[:], start=True, stop=True,
                         perf_mode=DR)


def _build_phase1(warm=0):
    """Per core: q/k/v = xs @ (16*W) for its 512-row x slice, fp8 DoubleRow.

    Three per-projection passes of 8 PSUM tiles (4 row-chunks x 2 halves) so
    each pass only waits on its own 1MB weight stream. Outputs bf16, 16x the
    true projection; host rescales and adds biases. Weight DMAs split across
    sync+gpsimd issuers; one output DMA per projection.
      xsT [128, 4, 2, 512]: [p, kk, t, r] = x[r, 128*(2kk+t)+p]
      w*  [128, 4, 2, 1024]: [p, kk, t, f] = 16*W[128*(2kk+t)+p, f]
      out [128, 4, 1024]: [p, i, f] = proj[i*128+p, f]
    """
    nc = bacc.Bacc(None, target_bir_lowering=False)
    pk_in = nc.dram_tensor("pk", [128, 56, 512], E4, kind="ExternalInput")
    outs = [nc.dram_tensor(n, [128, 4, D], BF16, kind="ExternalOutput")
            for n in ("q", "k", "v")]
    with TileContext(nc) as tc:
        with tc.tile_pool(name="inp", bufs=1) as inp, \
             tc.tile_pool(name="ob", bufs=2) as ob, \
             tc.tile_pool(name="ps", bufs=8, space="PSUM") as ps:
            _warmup(nc, inp, ps, "ps", warm)
            pk = inp.tile([128, 56, 512], E4)
            for a, b in ((0, 6), (6, 24), (24, 40), (40, 56)):
                nc.sync.dma_start(pk[:, a:b], pk_in[:, a:b])
            for w_i in range(3):
                pz = [ps.tile([128, 512], F32, name=f"p{w_i}_{n2}", tag="ps")
                      for n2 in range(8)]
                for kk in range(KK):
                    for i in range(R // 128):
                        lhsT = pk[:, 6 * kk : 6 * kk + 2, i * 128 : (i + 1) * 128]
                        for h in range(2):
                            if w_i == 0:
                                rhs = pk[:, 6 * kk + 2 + 2 * h : 6 * kk + 4 + 2 * h]
                            else:
                                c0 = 8 + 16 * w_i + 4 * kk + 2 * h
                                rhs = pk[:, c0 : c0 + 2]
                            nc.tensor.matmul(
                                pz[i * 2 + h][:],
                                lhsT,
                                rhs,
                                start=(kk == 0), stop=(kk == KK - 1),
                                perf_mode=DR,
                            )
                osb = ob.tile([128, 4, D], BF16, name=f"o{w_i}", tag="ob")
                for i in range(R // 128):
                    for h in range(2):
                        dst = osb[:, i, h * 512 : (h + 1) * 512]
                        if h == 0:
                            nc.vector.tensor_copy(dst, pz[i * 2 + h][:])
                        else:
                            nc.scalar.copy(dst, pz[i * 2 + h][:])
                nc.sync.dma_start(outs[w_i][:, 0:2], osb[:, 0:2])
                nc.sync.dma_start(outs[w_i][:, 2:4], osb[:, 2:4])
    nc.finalize()
    return nc


def _build_phase2(warm=0):
    """Per core: anti-causal attention for its 512 query rows (16-row strips
    s = 8k+c, k=0..31) vs all 4096 keys; exact-triangle sparsity with a
    uniform SPMD program (all per-core variation lives in input data).

    Scores for key tile j only cover query columns [0, 16*(j+2)) — strips at
    or below the diagonal; masking uses host thresholds th (iota <= th) fused
    with the fp8 quantize in one vector op. PV is out[queries, features] with
    stationary P^T key-pair chunks of partial width, accumulated descending so
    the first matmul zeroes the full PSUM region.
      qt [128, 4, 2, 512]: [p,kk,t,q] = qT[128*(2kk+t)+p, strip-ordered q]
      kt [128, 32, 4, 2, 128]: [p,j,kk,t,c] = kT[128*(2kk+t)+p, 128j+c]
      vi [128, 32, 1026]: [p,j,c] = v_aug[128j+p, c]
      th [128, 32]: #queries visible to key 128j+p, minus one
      rd [128, 4, 1026] bf16: [p,qc,:] = unnormalized read for query col
      qc*128+p (strip order) + denominator column.
    """
    nc = bacc.Bacc(None, target_bir_lowering=False)
    qt_in = nc.dram_tensor("qt", [128, KK, 2, R], E4, kind="ExternalInput")
    kt_in = nc.dram_tensor("kt", [128, J, KK, 2, 128], E4, kind="ExternalInput")
    v_in = nc.dram_tensor("vi", [128, J, VA], E4, kind="ExternalInput")
    thr = nc.dram_tensor("th", [128, J], F32, kind="ExternalInput")
    rd = nc.dram_tensor("rd", [128, 4, VA], BF16, kind="ExternalOutput")
    with TileContext(nc) as tc:
        with tc.tile_pool(name="cst", bufs=1) as cst, \
             tc.tile_pool(name="sp", bufs=2, space="PSUM") as sp, \
             tc.tile_pool(name="ep", bufs=3) as ep, \
             tc.tile_pool(name="p2", bufs=2, space="PSUM") as p2, \
             tc.tile_pool(name="no", bufs=4) as no:
            _warmup(nc, cst, sp, "s", warm)
            io = cst.tile([128, R], F32)
            nc.gpsimd.iota(io[:], [[1, R]], channel_multiplier=0,
                           allow_small_or_imprecise_dtypes=True)
            th = cst.tile([128, J], F32)
            nc.scalar.dma_start(th[:], thr[:])
            qt = cst.tile([128, KK, 2, R], E4)
            kt = cst.tile([128, J, KK, 2, 128], E4)
            vt = cst.tile([128, J, VA], E4)
            nc.sync.dma_start(qt[:], qt_in[:])
            for a, b in ((30, 32), (24, 30), (12, 24), (0, 12)):
                nc.sync.dma_start(kt[:, a:b], kt_in[:, a:b])
            for a, b in ((16, 32), (0, 16)):
                nc.sync.dma_start(vt[:, a:b], v_in[:, a:b])
            pt = cst.tile([128, J, R], E4)
            # ---- scores: S^T[key, q] on visible strips, exp, mask -> fp8 ----
            for j in range(J - 1, -1, -1):
                w = W_J[j]
                ps_ = sp.tile([128, 512], F32, name=f"s{j}", tag="s")
                for kk in range(KK):
                    nc.tensor.matmul(
                        ps_[:, 0:w],
                        kt[:, j, kk],
                        qt[:, kk, :, 0:w],
                        start=(kk == 0), stop=(kk == KK - 1),
                        perf_mode=DR,
                    )
                ex = ep.tile([128, 512], F32, name=f"e{j}", tag="e")
                nc.scalar.activation(ex[:, 0:w], ps_[:, 0:w],
                                     mybir.ActivationFunctionType.Exp,
                                     scale=float(1.0 / np.sqrt(D)))
                nc.vector.scalar_tensor_tensor(
                    pt[:, j, 0:w], io[:, 0:w], th[:, j : j + 1], ex[:, 0:w],
                    op0=mybir.AluOpType.is_le, op1=mybir.AluOpType.mult,
                )
            # ---- PV: read[q, f] over visible key pairs (descending) ----
            for qc in range(4):
                pz = [p2.tile([128, CH], F32, name=f"pv{qc}_{ch}", tag=f"ch{ch}")
                      for ch in range(3)]
                ms = list(range(J // 2 - 1, 4 * qc - 1, -1))
                for m in ms:
                    wd = min(128, 16 * (2 * m + 2) - 128 * qc)
                    lhsT = pt[:, 2 * m : 2 * m + 2, qc * 128 : qc * 128 + wd]
                    for ch in range(3):
                        nc.tensor.matmul(
                            pz[ch][0:wd, :],
                            lhsT,
                            vt[:, 2 * m : 2 * m + 2, ch * CH : (ch + 1) * CH],
                            start=(m == ms[0]), stop=(m == ms[-1]),
                            perf_mode=DR,
                            skip_group_check=True,
                        )
                o = no.tile([128, VA], BF16, name=f"rd{qc}", tag="rd")
                nc.vector.tensor_copy(o[:, 0:CH], pz[0][:])
                nc.scalar.copy(o[:, CH : 2 * CH], pz[1][:])
                nc.vector.tensor_copy(o[:, 2 * CH : VA], pz[2][:])
                nc.sync.dma_start(rd[:, qc, :], o[:])
    nc.finalize()
    return nc


def _dr_layout(aT):
    # [1024, C] (contraction-major) -> [128, 4, 2, C] DoubleRow layout
    c = aT.shape[1]
    return np.ascontiguousarray(aT.reshape(KK, 2, 128, c).transpose(2, 0, 1, 3))


def _strip_rows(c):
    # query rows owned by core c in on-device column order (16-row strips)
    q = np.arange(R)
    return 128 * (q // 16) + 16 * c + (q % 16)


def prep_phase1(x, Wq, Wk, Wv):
    xq = x.astype(E4NP)
    w_ins = [np.ascontiguousarray(
        _dr_layout((W * WSC).astype(E4NP).reshape(D, D))) for W in (Wq, Wk, Wv)]
    in_maps = []
    for c in range(NCORES):
        xsT = _dr_layout(np.ascontiguousarray(xq[c * R : (c + 1) * R].T))
        pk = np.zeros((128, 56, 512), E4NP)
        for kk in range(KK):
            for t in range(2):
                pk[:, 6 * kk + t] = xsT[:, kk, t]
                for h in range(2):
                    pk[:, 6 * kk + 2 + 2 * h + t] = w_ins[0][:, kk, t, h * 512 : (h + 1) * 512]
                    for w_i in (1, 2):
                        pk[:, 8 + 16 * w_i + 4 * kk + 2 * h + t] = \
                            w_ins[w_i][:, kk, t, h * 512 : (h + 1) * 512]
        in_maps.append({"pk": pk})
    return in_maps


def _p1out(res, name):
    # [128, 4, D] -> [512, D] float32
    return res[name].astype(np.float32).transpose(1, 0, 2).reshape(R, D)


def prep_phase2(res1, bq, bk, bv):
    inv = np.float32(1.0 / WSC)
    q_g = np.concatenate([_p1out(res1[c], "q") for c in range(NCORES)]) * inv + bq
    k_g = np.concatenate([_p1out(res1[c], "k") for c in range(NCORES)]) * inv + bk
    v_g = np.concatenate([_p1out(res1[c], "v") for c in range(NCORES)]) * inv + bv
    kT = np.ascontiguousarray(k_g.T.astype(E4NP))
    kt_in = np.ascontiguousarray(
        kT.reshape(KK, 2, 128, J, 128).transpose(2, 3, 0, 1, 4))
    v_aug = np.concatenate(
        [v_g, np.ones((S, 1), np.float32), np.zeros((S, 1), np.float32)], axis=1)
    v_in = np.ascontiguousarray(
        v_aug.astype(E4NP).reshape(J, 128, VA).transpose(1, 0, 2))
    qT8 = q_g.T.astype(E4NP)
    p_idx = np.arange(128)[:, None]
    j_idx = np.arange(J)[None, :]
    keys = (128 * j_idx + p_idx).ravel()
    in_maps = []
    for c in range(NCORES):
        rows = _strip_rows(c)
        qt = _dr_layout(np.ascontiguousarray(qT8[:, rows]))
        th_c = (np.searchsorted(rows, keys, side="right") - 1).reshape(128, J)
        in_maps.append({"qt": qt, "kt": kt_in, "vi": v_in,
                        "th": np.ascontiguousarray(th_c.astype(np.float32))})
    return in_maps


def finish(x, res2):
    read = np.empty((S, D), np.float32)
    for c in range(NCORES):
        r = res2[c]["rd"].astype(np.float32).transpose(1, 0, 2).reshape(R, VA)
        read[_strip_rows(c)] = r[:, :D] / r[:, D : D + 1]
    return np.concatenate([x, read], axis=1)


def kernel(x, Wk, bk, Wq, bq, Wv, bv):
    x = np.asarray(x, dtype=np.float32)
    Wk = np.asarray(Wk, dtype=np.float32)
    Wq = np.asarray(Wq, dtype=np.float32)
    Wv = np.asarray(Wv, dtype=np.float32)
    bk = np.asarray(bk, dtype=np.float32)
    bq = np.asarray(bq, dtype=np.float32)
    bv = np.asarray(bv, dtype=np.float32)

    if "p1" not in _cache:
        _cache["p1"] = _build_phase1()
    if "p2" not in _cache:
        _cache["p2"] = _build_phase2()

    in_maps1 = prep_phase1(x, Wq, Wk, Wv)
    res1 = run_bass_kernel_spmd(_cache["p1"], in_maps1, list(range(NCORES))).results
    in_maps2 = prep_phase2(res1, bq, bk, bv)
    res2 = run_bass_kernel_spmd(_cache["p2"], in_maps2, list(range(NCORES))).results
    return finish(x, res2)
